# revision 19
# baseline (speedup 1.0000x reference)
"""Trainium2 Bass kernel for the 16-qubit angle-encoder (nn_Encoder).

Math: out[b, k] = (1/256) * exp(i * sum_q s_q(k) * pi * x[b, q]) where
s_q(k) = +1 if bit (15-q) of k is set else -1.  Split k = hi*256 + lo:
the phase separates into phaseHi[b, hi] + phaseLo[b, lo], so each output
row is a complex outer product of two 256-entry tables U[b, hi], W[b, lo].
Each core handles 32 batch rows (pure data parallel over 8 cores).

The kernel is store-bandwidth-bound, so the device does ONLY the
bandwidth-critical outer-product expansion and stores the state vector
in fp16 (|out_k| = 1/256 for every k, so fp16 keeps ~2^-11 relative
accuracy; the host widens to complex64 while unsharding).  The tiny
tables (32x256 complex sin/cos values per core, 0.1% of the output
work) are precomputed on the host and shipped pre-laid-out for the PE:

  tabs[2, 32*768] bf16, per row b the 768 columns are
    [ Ur[b, 0:256] | W0[b, 0:512] ]   row 0
    [ Ui[b, 0:256] | W1[b, 0:512] ]   row 1
  with W0[2*lo+c] = (Wr, Wi)[c],  W1[2*lo+c] = (-Wi, Wr)[c], so a K=2
  bf16 matmul emits an interleaved re/im [128, 512] fp32 block:
    out[hi, 2*lo+c] = Ur*W0 + Ui*W1  ->  re = Ur*Wr - Ui*Wi,
                                         im = Ur*Wi + Ui*Wr.

Device pipeline per row b (x2 chunks of 128 hi values):
  K=2 matmul -> PSUM [128, 512] fp32 -> fp16 copy to SBUF (alternating
  vector/scalar engines) -> grouped 1 MiB contiguous HBM stores.  The
  DRAM y layout is chunk-major [c, p, b, j] so each store descriptor
  covers a multi-row contiguous run per partition; first groups are
  small so the store stream starts as early as possible.
"""

import sys

sys.path.insert(0, "/opt/trn_rl_repo")

import numpy as np
import ml_dtypes

BF16 = ml_dtypes.bfloat16
N_QUBITS = 16
BATCH = 256
N_CORES = 8
B_PER_CORE = BATCH // N_CORES  # 32

_COMPILED = {}
_GROUP_ROWS = (1, 2, 2, 2, 3, 3, 4, 4, 4, 4, 3)
_SPLIT1 = 0
_RAMP_SPLIT = 0
_PE_WARM = 0


def _core_tables(x: np.ndarray) -> np.ndarray:
    """[32, 16] fp32 -> [2, 32*768] bf16 table block for one core."""
    h = np.pi * x.astype(np.float64)
    j = np.arange(256)
    q = np.arange(8)[:, None]
    sgn = 2.0 * ((j[None, :] >> (7 - q)) & 1) - 1.0  # [8, 256]
    U = np.exp(1j * (h[:, 0:8] @ sgn)) / 256.0  # [32, 256]
    W = np.exp(1j * (h[:, 8:16] @ sgn))  # [32, 256]
    t = np.empty((2, B_PER_CORE, 768), np.float32)
    t[0, :, 0:256] = U.real
    t[1, :, 0:256] = U.imag
    t[0, :, 256:768:2] = W.real
    t[0, :, 257:768:2] = W.imag
    t[1, :, 256:768:2] = -W.imag
    t[1, :, 257:768:2] = W.real
    return np.ascontiguousarray(t.reshape(2, -1).astype(BF16))


def _build_module(n_rep: int = 1, full_rep: bool = False):
    import concourse.bacc as bacc
    import concourse.tile as tile
    import concourse.mybir as mybir

    fp32 = mybir.dt.float32
    fp16 = mybir.dt.float16
    bf16 = mybir.dt.bfloat16

    nc = bacc.Bacc("TRN2", target_bir_lowering=False, debug=False,
                   num_devices=N_CORES)
    tabs_in = nc.declare_dram_parameter("tabs", [2, B_PER_CORE * 768], bf16,
                                        isOutput=False)
    # chunk-major [c, p, b, j]: out[b, (c*128+p)*512 + j] = y[c, p, b, j];
    # host transposes while unsharding (gives multi-row contiguous DRAM
    # runs per partition for the grouped stores)
    y_out = nc.declare_dram_parameter("y", [2, 128, B_PER_CORE, 512], fp16,
                                      isOutput=True)
    B = B_PER_CORE

    with tile.TileContext(nc) as tc:
        with (
            tc.tile_pool(name="tables", bufs=1) as tp,
            tc.tile_pool(name="stage", bufs=8) as sp,
            tc.tile_pool(name="psum", bufs=8, space="PSUM") as pp,
        ):
            if _PE_WARM:
                # dummy matmuls on a zeroed tile start the PE p-state ramp
                # clock early, so ramp-phase matmuls reach max clock sooner
                warm = tp.tile([2, 640], bf16)
                nc.vector.memset(warm[:], 0.0)
                for _ in range(_PE_WARM):
                    wps = pp.tile([128, 512], fp32, tag="ps")
                    nc.tensor.matmul(wps[:], warm[:, 0:128],
                                     warm[:, 128:640], start=True, stop=True)

            tabs = tp.tile([2, B * 768], bf16)
            # Graded loads: row 0 first so the stream starts immediately,
            # then rows 1-3, then the bulk.
            nc.sync.dma_start(tabs[:, 0:768], tabs_in[:, 0:768])
            nc.scalar.dma_start(tabs[:, 768:3072], tabs_in[:, 768:3072])
            nc.sync.dma_start(tabs[:, 3072:B * 768], tabs_in[:, 3072:B * 768])

            # Store groups: small first so the DMA stream starts as soon as
            # one block is staged, then 1 MiB (4 rows) per dma_start.
            group_rows = list(_GROUP_ROWS)
            assert sum(group_rows) == B

            # PSUM->SBUF copies alternate between the only two engines that
            # can read PSUM (DVE / ACT); together they outrun the store DMA.
            copy_eng = [nc.vector.tensor_copy,
                        lambda d, s: nc.scalar.copy(d, s)]

            for _rep in range(n_rep):
                blk = 0
                b0 = 0
                for gi, rows in enumerate(group_rows):
                    st = sp.tile([128, 1024 * rows], fp16, tag="st")
                    for db in range(rows):
                        b = b0 + db
                        rhs = tabs[0:2, b * 768 + 256:b * 768 + 768]
                        for chunk in range(2):
                            off = b * 768 + chunk * 128
                            ps = pp.tile([128, 512], fp32, tag="ps")
                            # chunk-major staging to match the DRAM layout
                            col = (chunk * rows + db) * 512
                            dst = st[:, col:col + 512]
                            if gi == 0 and db == 0 and chunk == 0 and _SPLIT1:
                                # split the very first block by output column
                                # so the first (tiny) store launches as early
                                # as possible
                                s = _SPLIT1
                                nc.tensor.matmul(ps[:, 0:s],
                                                 tabs[0:2, off:off + 128],
                                                 rhs[:, 0:s],
                                                 start=True, stop=True)
                                nc.vector.tensor_copy(dst[:, 0:s], ps[:, 0:s])
                                nc.sync.dma_start(y_out[chunk, :, b, 0:s],
                                                  dst[:, 0:s])
                                nc.tensor.matmul(ps[:, s:512],
                                                 tabs[0:2, off:off + 128],
                                                 rhs[:, s:512],
                                                 start=True, stop=True)
                                nc.scalar.copy(dst[:, s:512], ps[:, s:512])
                                nc.sync.dma_start(y_out[chunk, :, b, s:512],
                                                  dst[:, s:512])
                                blk += 1
                                continue
                            nc.tensor.matmul(ps[:], tabs[0:2, off:off + 128],
                                             rhs, start=True, stop=True)
                            if blk < _RAMP_SPLIT:
                                # halve block latency during the pipeline
                                # ramp: both PSUM-reader engines copy one
                                # column half in parallel
                                nc.vector.tensor_copy(dst[:, 0:256],
                                                      ps[:, 0:256])
                                nc.scalar.copy(dst[:, 256:512],
                                               ps[:, 256:512])
                            else:
                                copy_eng[blk % 2](dst, ps[:])
                            blk += 1
                            if gi == 0:
                                # very first row: store each block as soon
                                # as its copy lands
                                nc.sync.dma_start(y_out[chunk, :, b, :], dst)
                    if gi > 0:
                        # alternate stores across the two HWDGE rings
                        # (SP via nc.sync, ACT via nc.scalar) so descriptor
                        # generation for consecutive stores overlaps
                        eng = nc.sync if gi % 2 == 0 else nc.scalar
                        eng.dma_start(
                            y_out[:, :, b0:b0 + rows, :].rearrange(
                                "c p b j -> p c b j"),
                            st[:])
                    b0 += rows

    nc.compile()
    return nc


def _get_compiled(n_rep: int = 1, full_rep: bool = False):
    key = ("nc", n_rep, full_rep)
    if key not in _COMPILED:
        _COMPILED[key] = _build_module(n_rep, full_rep)
    return _COMPILED[key]


def _make_inputs(x: np.ndarray) -> list:
    return [
        {"tabs": _core_tables(x[c * B_PER_CORE:(c + 1) * B_PER_CORE])}
        for c in range(N_CORES)
    ]


def _run(inputs: np.ndarray, trace: bool = False):
    from concourse.bass_utils import run_bass_kernel_spmd

    nc = _get_compiled()
    x = np.asarray(inputs, dtype=np.float32)
    assert x.shape == (BATCH, N_QUBITS)
    in_maps = _make_inputs(x)
    res = run_bass_kernel_spmd(nc, in_maps, core_ids=list(range(N_CORES)),
                               trace=trace)
    parts = []
    for c in range(N_CORES):
        y = np.asarray(res.results[c]["y"])  # [2, 128, 32, 512] fp16
        y = np.transpose(y, (2, 0, 1, 3)).reshape(B_PER_CORE, 2 ** 17)
        parts.append(y.astype(np.float32).view(np.complex64))
    out = np.concatenate(parts, axis=0)
    return out, res


def kernel(inputs: np.ndarray) -> np.ndarray:
    out, _ = _run(inputs, trace=False)
    return out


# revision 20
# speedup vs baseline: 1.0016x; 1.0016x over previous
"""Trainium2 Bass kernel for the 16-qubit angle-encoder (nn_Encoder).

Math: out[b, k] = (1/256) * exp(i * sum_q s_q(k) * pi * x[b, q]) where
s_q(k) = +1 if bit (15-q) of k is set else -1.  Split k = hi*256 + lo:
the phase separates into phaseHi[b, hi] + phaseLo[b, lo], so each output
row is a complex outer product of two 256-entry tables U[b, hi], W[b, lo].
Each core handles 32 batch rows (pure data parallel over 8 cores).

The kernel is store-bandwidth-bound, so the device does ONLY the
bandwidth-critical outer-product expansion and stores the state vector
in fp16 (|out_k| = 1/256 for every k, so fp16 keeps ~2^-11 relative
accuracy; the host widens to complex64 while unsharding).  The tiny
tables (32x256 complex sin/cos values per core, 0.1% of the output
work) are precomputed on the host and shipped pre-laid-out for the PE:

  tabs[2, 32*768] bf16, per row b the 768 columns are
    [ Ur[b, 0:256] | W0[b, 0:512] ]   row 0
    [ Ui[b, 0:256] | W1[b, 0:512] ]   row 1
  with W0[2*lo+c] = (Wr, Wi)[c],  W1[2*lo+c] = (-Wi, Wr)[c], so a K=2
  bf16 matmul emits an interleaved re/im [128, 512] fp32 block:
    out[hi, 2*lo+c] = Ur*W0 + Ui*W1  ->  re = Ur*Wr - Ui*Wi,
                                         im = Ur*Wi + Ui*Wr.

Device pipeline per row b (x2 chunks of 128 hi values):
  K=2 matmul -> PSUM [128, 512] fp32 -> fp16 copy to SBUF (alternating
  vector/scalar engines) -> grouped 1 MiB contiguous HBM stores.  The
  DRAM y layout is chunk-major [c, p, b, j] so each store descriptor
  covers a multi-row contiguous run per partition; first groups are
  small so the store stream starts as early as possible.
"""

import sys

sys.path.insert(0, "/opt/trn_rl_repo")

import numpy as np
import ml_dtypes

BF16 = ml_dtypes.bfloat16
N_QUBITS = 16
BATCH = 256
N_CORES = 8
B_PER_CORE = BATCH // N_CORES  # 32

_COMPILED = {}
_GROUP_ROWS = (1, 2, 2, 2, 3, 3, 4, 4, 4, 4, 2, 1)
_SPLIT1 = 0
_RAMP_SPLIT = 0
_PE_WARM = 0


def _core_tables(x: np.ndarray) -> np.ndarray:
    """[32, 16] fp32 -> [2, 32*768] bf16 table block for one core."""
    h = np.pi * x.astype(np.float64)
    j = np.arange(256)
    q = np.arange(8)[:, None]
    sgn = 2.0 * ((j[None, :] >> (7 - q)) & 1) - 1.0  # [8, 256]
    U = np.exp(1j * (h[:, 0:8] @ sgn)) / 256.0  # [32, 256]
    W = np.exp(1j * (h[:, 8:16] @ sgn))  # [32, 256]
    t = np.empty((2, B_PER_CORE, 768), np.float32)
    t[0, :, 0:256] = U.real
    t[1, :, 0:256] = U.imag
    t[0, :, 256:768:2] = W.real
    t[0, :, 257:768:2] = W.imag
    t[1, :, 256:768:2] = -W.imag
    t[1, :, 257:768:2] = W.real
    return np.ascontiguousarray(t.reshape(2, -1).astype(BF16))


def _build_module(n_rep: int = 1, full_rep: bool = False):
    import concourse.bacc as bacc
    import concourse.tile as tile
    import concourse.mybir as mybir

    fp32 = mybir.dt.float32
    fp16 = mybir.dt.float16
    bf16 = mybir.dt.bfloat16

    nc = bacc.Bacc("TRN2", target_bir_lowering=False, debug=False,
                   num_devices=N_CORES)
    tabs_in = nc.declare_dram_parameter("tabs", [2, B_PER_CORE * 768], bf16,
                                        isOutput=False)
    # chunk-major [c, p, b, j]: out[b, (c*128+p)*512 + j] = y[c, p, b, j];
    # host transposes while unsharding (gives multi-row contiguous DRAM
    # runs per partition for the grouped stores)
    y_out = nc.declare_dram_parameter("y", [2, 128, B_PER_CORE, 512], fp16,
                                      isOutput=True)
    B = B_PER_CORE

    with tile.TileContext(nc) as tc:
        with (
            tc.tile_pool(name="tables", bufs=1) as tp,
            tc.tile_pool(name="stage", bufs=8) as sp,
            tc.tile_pool(name="psum", bufs=8, space="PSUM") as pp,
        ):
            if _PE_WARM:
                # dummy matmuls on a zeroed tile start the PE p-state ramp
                # clock early, so ramp-phase matmuls reach max clock sooner
                warm = tp.tile([2, 640], bf16)
                nc.vector.memset(warm[:], 0.0)
                for _ in range(_PE_WARM):
                    wps = pp.tile([128, 512], fp32, tag="ps")
                    nc.tensor.matmul(wps[:], warm[:, 0:128],
                                     warm[:, 128:640], start=True, stop=True)

            tabs = tp.tile([2, B * 768], bf16)
            # Graded loads: row 0 first so the stream starts immediately,
            # then rows 1-3, then the bulk.
            nc.sync.dma_start(tabs[:, 0:768], tabs_in[:, 0:768])
            nc.scalar.dma_start(tabs[:, 768:3072], tabs_in[:, 768:3072])
            nc.sync.dma_start(tabs[:, 3072:B * 768], tabs_in[:, 3072:B * 768])

            # Store groups: small first so the DMA stream starts as soon as
            # one block is staged, then 1 MiB (4 rows) per dma_start.
            group_rows = list(_GROUP_ROWS)
            assert sum(group_rows) == B

            # PSUM->SBUF copies alternate between the only two engines that
            # can read PSUM (DVE / ACT); together they outrun the store DMA.
            copy_eng = [nc.vector.tensor_copy,
                        lambda d, s: nc.scalar.copy(d, s)]

            for _rep in range(n_rep):
                blk = 0
                b0 = 0
                for gi, rows in enumerate(group_rows):
                    st = sp.tile([128, 1024 * rows], fp16, tag="st")
                    for db in range(rows):
                        b = b0 + db
                        rhs = tabs[0:2, b * 768 + 256:b * 768 + 768]
                        for chunk in range(2):
                            off = b * 768 + chunk * 128
                            ps = pp.tile([128, 512], fp32, tag="ps")
                            # chunk-major staging to match the DRAM layout
                            col = (chunk * rows + db) * 512
                            dst = st[:, col:col + 512]
                            if gi == 0 and db == 0 and chunk == 0 and _SPLIT1:
                                # split the very first block by output column
                                # so the first (tiny) store launches as early
                                # as possible
                                s = _SPLIT1
                                nc.tensor.matmul(ps[:, 0:s],
                                                 tabs[0:2, off:off + 128],
                                                 rhs[:, 0:s],
                                                 start=True, stop=True)
                                nc.vector.tensor_copy(dst[:, 0:s], ps[:, 0:s])
                                nc.sync.dma_start(y_out[chunk, :, b, 0:s],
                                                  dst[:, 0:s])
                                nc.tensor.matmul(ps[:, s:512],
                                                 tabs[0:2, off:off + 128],
                                                 rhs[:, s:512],
                                                 start=True, stop=True)
                                nc.scalar.copy(dst[:, s:512], ps[:, s:512])
                                nc.sync.dma_start(y_out[chunk, :, b, s:512],
                                                  dst[:, s:512])
                                blk += 1
                                continue
                            nc.tensor.matmul(ps[:], tabs[0:2, off:off + 128],
                                             rhs, start=True, stop=True)
                            if blk < _RAMP_SPLIT:
                                # halve block latency during the pipeline
                                # ramp: both PSUM-reader engines copy one
                                # column half in parallel
                                nc.vector.tensor_copy(dst[:, 0:256],
                                                      ps[:, 0:256])
                                nc.scalar.copy(dst[:, 256:512],
                                               ps[:, 256:512])
                            else:
                                copy_eng[blk % 2](dst, ps[:])
                            blk += 1
                            if gi == 0:
                                # very first row: store each block as soon
                                # as its copy lands
                                nc.sync.dma_start(y_out[chunk, :, b, :], dst)
                    if gi > 0:
                        # alternate stores across the two HWDGE rings
                        # (SP via nc.sync, ACT via nc.scalar) so descriptor
                        # generation for consecutive stores overlaps
                        eng = nc.sync if gi % 2 == 0 else nc.scalar
                        eng.dma_start(
                            y_out[:, :, b0:b0 + rows, :].rearrange(
                                "c p b j -> p c b j"),
                            st[:])
                    b0 += rows

    nc.compile()
    return nc


def _get_compiled(n_rep: int = 1, full_rep: bool = False):
    key = ("nc", n_rep, full_rep)
    if key not in _COMPILED:
        _COMPILED[key] = _build_module(n_rep, full_rep)
    return _COMPILED[key]


def _make_inputs(x: np.ndarray) -> list:
    return [
        {"tabs": _core_tables(x[c * B_PER_CORE:(c + 1) * B_PER_CORE])}
        for c in range(N_CORES)
    ]


def _run(inputs: np.ndarray, trace: bool = False):
    from concourse.bass_utils import run_bass_kernel_spmd

    nc = _get_compiled()
    x = np.asarray(inputs, dtype=np.float32)
    assert x.shape == (BATCH, N_QUBITS)
    in_maps = _make_inputs(x)
    res = run_bass_kernel_spmd(nc, in_maps, core_ids=list(range(N_CORES)),
                               trace=trace)
    parts = []
    for c in range(N_CORES):
        y = np.asarray(res.results[c]["y"])  # [2, 128, 32, 512] fp16
        y = np.transpose(y, (2, 0, 1, 3)).reshape(B_PER_CORE, 2 ** 17)
        parts.append(y.astype(np.float32).view(np.complex64))
    out = np.concatenate(parts, axis=0)
    return out, res


def kernel(inputs: np.ndarray) -> np.ndarray:
    out, _ = _run(inputs, trace=False)
    return out
